# revision 16
# baseline (speedup 1.0000x reference)
"""Trainium2 Bass kernel for nn_CalibrationNetwork (MoE-routed 3-layer MLP + softmax).

Strategy (v3): judge-contiguous scheduling
------------------------------------------
Host sorts samples by judge. The 32 judges are ranked by size and snake-
assigned to 8 cores x 4 slots; slot k has a common (max-padded) pair count
across cores so one SPMD program serves all cores. Per-judge combined
weights (W1+W1_a etc.) are packed per slot; all matmuls then use N=512
moving columns so LDWEIGHTS hides in the PE reorder window.

Layouts (parity-pair packing, 2 samples per column everywhere):
  z partition   p = par*64 + h           (par = sample parity in its pair)
  L1: stationary per q = [12 rows=(par,d0..5), 128 cols=(par,h)] block-diag,
      7 q's stacked on disjoint row ranges QROW[q] (2 per 32-strip) so all
      share one 128-col block; x lives on the same rows. Row-tiled matmuls
      (tile_position=(32g,0)) run concurrently across strips.
  L2: stationary [128=(par,h1), 128=(par,h2)] block-diag; bias b2 applied
      by the ACT relu2 drain (f32 bias columns).
  L3: stationary per q = V block [128=(par,h2), 10=(par,o)], col-tiled
      (tile_position=(0,32g)); output partitions 32g+par*5+o.
  V/b3 bias and softmax are applied on the HOST (judge known per sample).

Pipeline: chunks of <=512 pairs flow L1 -> relu1(DVE) -> L2 -> relu2(ACT)
-> L3 -> copy(ACT) with software-pipelined emission (L1 of chunk i+1 is
emitted before L2 of chunk i, etc.) and two 2-bank PSUM tags as the
pipeline buffer. A short warmup matmul burst trips the PE HAM throttle
to full clock while the input DMAs land.
"""

import numpy as np
import ml_dtypes

B, J, Q, O, H = 32768, 32, 7, 5, 64
N_CORES = 8
NSLOTS = J // N_CORES          # 4 judges (slots) per core
CHUNK = 512                    # max pairs per matmul
PADP = 32                      # slot pair counts padded to multiple of this
WJ = 456                       # weight cols/slot: 2x128 L1 + 128 L2 + 70 V + 2 b2
QROW = (0, 32, 64, 96, 0, 32, 64)    # x/W1 row base per question (32-aligned)
XB_OF_Q = (0, 0, 0, 0, 1, 1, 1)      # x/W1 column block per question
G_OF_Q = (0, 1, 2, 3, 0, 1, 2)       # 32-strip (row/col group) per question

_bf16 = ml_dtypes.bfloat16
_cache = {}


def _chunks(S, ragged_first=False):
    """Split S pairs into matmul chunks (512s + one ragged multiple of 32)."""
    out = [CHUNK] * (S // CHUNK)
    if S % CHUNK:
        if ragged_first:
            out = [S % CHUNK] + out
        else:
            out.append(S % CHUNK)
    return out


def _chunk_plan(slots):
    """Per-slot chunk lists with start/end taper for pipeline fill/drain."""
    plan = []
    for s, S in enumerate(slots):
        ch = _chunks(S, ragged_first=(s == 0))
        plan.append(ch)
    # taper the very last chunk so the pipeline drain-out is short
    last = plan[-1]
    if last and last[-1] > 256:
        m = last.pop()
        last.extend([m - 128, 64, 64])
    elif last and last[-1] > 128:
        m = last.pop()
        last.extend([m - 64, 64])
    return plan


# ----------------------------------------------------------------------------
# device program
# ----------------------------------------------------------------------------

def _build_program(slots):
    import concourse.bacc as bacc
    import concourse.tile as tile
    import concourse.mybir as mybir
    import concourse.bass as bass
    from contextlib import ExitStack

    bf = mybir.dt.bfloat16
    f32 = mybir.dt.float32
    AF = mybir.ActivationFunctionType

    TP = sum(slots)
    offs = np.cumsum([0] + list(slots))[:-1]

    nc = bacc.Bacc("TRN2", target_bir_lowering=False, debug=False)
    xa_d = nc.dram_tensor("xa", (128, 2 * TP), bf, kind="ExternalInput")
    wt_d = nc.dram_tensor("wt", (128, NSLOTS * WJ), bf, kind="ExternalInput")
    out_d = nc.dram_tensor("out", (128, 2 * TP), bf, kind="ExternalOutput")

    with ExitStack() as ctx:
        tc = ctx.enter_context(tile.TileContext(nc))
        cpool = ctx.enter_context(tc.tile_pool(name="const", bufs=1))
        ppool = ctx.enter_context(tc.tile_pool(name="ps", bufs=2, space="PSUM"))

        xa_t = cpool.tile([128, 2 * TP], bf)
        wt_t = cpool.tile([128, NSLOTS * WJ], bf)
        z1 = cpool.tile([128, 7 * TP], bf)
        z2 = cpool.tile([128, 7 * TP], bf)
        lg = cpool.tile([128, 2 * TP], bf)
        warm = cpool.tile([1, 8], bf)

        # preload the ACT Relu table during the startup window
        nc.vector.memset(warm[:], 1.0)
        nc.scalar.activation(warm[0:1, 0:1], warm[0:1, 1:2], AF.Relu, scale=1.0)
        # weights first (small), then x per slot so L1 of slot 0 starts early
        nc.sync.dma_start(wt_t[:], wt_d.ap())
        for s in range(NSLOTS):
            o = int(offs[s])
            for xb in range(2):
                nc.sync.dma_start(
                    xa_t[:, xb * TP + o:xb * TP + o + slots[s]],
                    xa_d.ap()[:, xb * TP + o:xb * TP + o + slots[s]])

        # chunk sequence: (slot, pair0, npairs)
        seq = []
        plan = _chunk_plan(slots)
        for s in range(NSLOTS):
            p0 = 0
            for n in plan[s]:
                seq.append((s, p0, n))
                p0 += n

        def drain3(engine_op, t, nb, n, dst, **kw):
            """Drain nb banks of n cols each from tile t into contiguous dst."""
            if n == 512:
                engine_op(dst, t[:, 0:nb * 512], **kw)
            else:
                src = bass.AP(t[:].tensor, t[:].offset,
                              [list(t[:].ap[0]), [512, nb], [1, n]])
                d = bass.AP(dst.tensor, dst.offset,
                            [list(dst.ap[0]), [n, nb], [1, n]])
                engine_op(d, src, **kw)

        def l1a(i):
            s, p0, n = seq[i]
            o = int(offs[s])
            wc = s * WJ
            zb = 7 * o + 7 * p0
            t = ppool.tile([128, 2048], f32, tag="pa", name=f"p1_{i}", bufs=1)
            for k, q in enumerate((0, 1, 2, 3)):
                rw = QROW[q]
                xb = XB_OF_Q[q]
                nc.tensor.matmul(
                    t[:, 512 * k:512 * k + n],
                    wt_t[rw:rw + 12, wc + 128 * xb:wc + 128 * xb + 128],
                    xa_t[rw:rw + 12, xb * TP + o + p0:xb * TP + o + p0 + n],
                    start=True, stop=True,
                    tile_position=(32 * G_OF_Q[q], 0))
            drain3(nc.vector.tensor_scalar_max, t, 4, n,
                   z1[:, zb:zb + 4 * n], scalar1=0.0)
            return t

        def l1b(i, t):
            s, p0, n = seq[i]
            o = int(offs[s])
            wc = s * WJ
            zb = 7 * o + 7 * p0
            for k, q in enumerate((4, 5, 6)):
                rw = QROW[q]
                xb = XB_OF_Q[q]
                nc.tensor.matmul(
                    t[:, 512 * k:512 * k + n],
                    wt_t[rw:rw + 12, wc + 128 * xb:wc + 128 * xb + 128],
                    xa_t[rw:rw + 12, xb * TP + o + p0:xb * TP + o + p0 + n],
                    start=True, stop=True,
                    tile_position=(32 * G_OF_Q[q], 0))
            drain3(nc.vector.tensor_scalar_max, t, 3, n,
                   z1[:, zb + 4 * n:zb + 7 * n], scalar1=0.0)

        def l2(i, half):
            """Emit half 0 (first ~2 pieces) or half 1 (rest) of chunk i's L2."""
            s, p0, n = seq[i]
            o = int(offs[s])
            wc = s * WJ
            zb = 7 * o + 7 * p0
            b2ap = wt_t[:, wc + 454:wc + 456].bitcast(f32)
            bounds = [0, min(2048, 7 * n)] if half == 0 else [min(2048, 7 * n), 7 * n]
            done = bounds[0]
            while done < bounds[1]:
                piece = min(1024, bounds[1] - done)
                t = ppool.tile([128, 1024], f32, tag="pb", name=f"p2_{i}_{done}")
                na = min(512, piece)
                nc.tensor.matmul(t[:, 0:na], wt_t[:, wc + 256:wc + 384],
                                 z1[:, zb + done:zb + done + na],
                                 start=True, stop=True)
                if piece > 512:
                    nc.tensor.matmul(t[:, 512:piece], wt_t[:, wc + 256:wc + 384],
                                     z1[:, zb + done + 512:zb + done + piece],
                                     start=True, stop=True)
                if piece <= 512 or piece == 1024:
                    # contiguous in both PSUM and z2
                    nc.scalar.activation(z2[:, zb + done:zb + done + piece],
                                         t[:, 0:piece], AF.Relu, bias=b2ap, scale=1.0)
                else:
                    # unequal halves: two drains
                    nc.scalar.activation(z2[:, zb + done:zb + done + 512],
                                         t[:, 0:512], AF.Relu, bias=b2ap, scale=1.0)
                    nc.scalar.activation(z2[:, zb + done + 512:zb + done + piece],
                                         t[:, 512:piece], AF.Relu, bias=b2ap, scale=1.0)
                done += piece

        def l3(i):
            s, p0, n = seq[i]
            o = int(offs[s])
            wc = s * WJ
            zb = 7 * o + 7 * p0
            lgb = 2 * o + 2 * p0
            t = ppool.tile([128, 1024], f32, tag="pb", name=f"p3_{i}")
            for q in range(7):
                g = G_OF_Q[q]
                rnd = 0 if q < 4 else 1
                nc.tensor.matmul(
                    t[32 * g:32 * g + 10, 512 * rnd:512 * rnd + n],
                    wt_t[:, wc + 384 + 10 * q:wc + 394 + 10 * q],
                    z2[:, zb + q * n:zb + (q + 1) * n],
                    start=True, stop=True,
                    tile_position=(0, 32 * g))
            drain3(nc.scalar.copy, t, 2, n, lg[:, lgb:lgb + 2 * n])
            # per-chunk output store; last one on the SP ring
            eng = nc.sync if i == len(seq) - 1 else nc.gpsimd
            eng.dma_start(out_d.ap()[:, lgb:lgb + 2 * n], lg[:, lgb:lgb + 2 * n])

        # software-pipelined emission: L1(i) | L2(i-1) | L3(i-2), with L2
        # pieces split around L1's second round to fill PE stalls
        nseq = len(seq)
        for i in range(nseq + 2):
            t = l1a(i) if i < nseq else None
            if 1 <= i <= nseq:
                l2(i - 1, 0)
            if t is not None:
                l1b(i, t)
            if 1 <= i <= nseq:
                l2(i - 1, 1)
            if 2 <= i:
                l3(i - 2)

    nc.compile()
    return nc


def _get_program(slots):
    key = tuple(slots)
    if key not in _cache:
        _cache[key] = _build_program(key)
    return _cache[key]


# ----------------------------------------------------------------------------
# host-side scheduling and packing
# ----------------------------------------------------------------------------

def _schedule(judge_ids):
    """Snake-assign judges to (core, slot); returns slot sizes + per-core judge
    lists + per-judge sample index arrays (sorted order)."""
    jid = np.asarray(judge_ids).astype(np.int64).ravel()
    assert jid.shape[0] == B
    order = np.argsort(jid, kind="stable")
    counts = np.bincount(jid, minlength=J)
    pos = np.cumsum([0] + list(counts))
    samples = [order[pos[j]:pos[j + 1]] for j in range(J)]
    pairs = np.array([(c + 1) // 2 for c in counts])

    rank = np.argsort(-pairs, kind="stable")
    slots = []
    assign = np.zeros((N_CORES, NSLOTS), np.int64)   # judge id per (core, slot)
    for s in range(NSLOTS):
        grp = rank[8 * s:8 * s + 8]
        size = int(-(-max(1, pairs[grp].max()) // PADP) * PADP)
        slots.append(size)
        for k in range(N_CORES):
            assign[k, s] = grp[k]
    return tuple(slots), assign, samples, pairs


def _pack_inputs(x, judge_ids, W1c, W2c, Vc, slots, assign, samples):
    TP = sum(slots)
    offs = np.cumsum([0] + list(slots))[:-1]
    x = np.asarray(x, np.float32)

    # weights per judge, packed per (core, slot)
    wtj = np.zeros((J, 128, WJ), np.float32)
    for q in range(Q):
        rw = QROW[q]
        xb = XB_OF_Q[q]
        for par in range(2):
            # L1 block: rows rw+par*6+d, cols 128*xb + par*64+h
            blk = W1c[:, q].transpose(0, 2, 1)          # [J, d, h]
            wtj[:, rw + 6 * par:rw + 6 * par + 6,
                128 * xb + 64 * par:128 * xb + 64 * par + 64] = blk
    for par in range(2):
        s = slice(64 * par, 64 * par + 64)
        wtj[:, s, 256 + 64 * par:256 + 64 * par + 64] = \
            W2c[:, :, 1:].transpose(0, 2, 1)            # rows h1, cols h2
        for q in range(Q):
            wtj[:, s, 384 + 10 * q + 5 * par:384 + 10 * q + 5 * par + 5] = \
                Vc[:, q, :, 1:].transpose(0, 2, 1)      # rows h2, cols o
    wt16 = np.zeros((J, 128, WJ), np.uint16)
    wt16[:, :, :454] = wtj[:, :, :454].astype(_bf16).view(np.uint16)
    b2 = np.concatenate([W2c[:, :, 0], W2c[:, :, 0]], axis=1)   # [J, 128]
    wt16[:, :, 454:456] = b2.astype(np.float32).view(np.uint16).reshape(J, 128, 2)

    in_maps = []
    for k in range(N_CORES):
        xa = np.zeros((128, 2 * TP), np.float32)
        wt = np.zeros((128, NSLOTS * WJ), np.uint16)
        for s in range(NSLOTS):
            j = int(assign[k, s])
            o = int(offs[s])
            g = samples[j]
            wt[:, s * WJ:(s + 1) * WJ] = wt16[j]
            for par in range(2):
                gs = g[par::2]
                ns = len(gs)
                xv = x[gs]                               # [ns, Q, O]
                for q in range(Q):
                    rw = QROW[q] + 6 * par
                    cb = XB_OF_Q[q] * TP + o
                    xa[rw, cb:cb + ns] = 1.0
                    xa[rw + 1:rw + 6, cb:cb + ns] = xv[:, q, :].T
        in_maps.append({"xa": xa.astype(_bf16),
                        "wt": wt.view(_bf16)})
    return in_maps


def _unpack_output(results, judge_ids, b3, slots, assign, samples):
    TP = sum(slots)
    offs = np.cumsum([0] + list(slots))[:-1]
    logits = np.zeros((B, Q, O), np.float32)
    for k in range(N_CORES):
        blob = np.asarray(results[k]["out"]).view(np.uint16)
        f = (blob.astype(np.uint32) << 16).view(np.float32)   # [128, 2*TP]
        for s in range(NSLOTS):
            j = int(assign[k, s])
            o = int(offs[s])
            g = samples[j]
            S = len(g)
            p0 = 0
            for n in _chunk_plan(slots)[s]:
                for q in range(Q):
                    gq = G_OF_Q[q]
                    rnd = 0 if q < 4 else 1
                    cb = 2 * o + 2 * p0 + rnd * n
                    for par in range(2):
                        rows = slice(32 * gq + 5 * par, 32 * gq + 5 * par + 5)
                        idx = 2 * (p0 + np.arange(n)) + par
                        v = idx < S
                        if v.any():
                            logits[g[idx[v]], q, :] = f[rows, cb:cb + n].T[v]
                p0 += n
    logits += b3[np.asarray(judge_ids).astype(np.int64).ravel()]
    m = logits.max(-1, keepdims=True)
    e = np.exp(logits - m)
    return (e / e.sum(-1, keepdims=True)).astype(np.float32)


def _prepare(x, judge_ids, W1, W1_a, W2, W2_a, V, V_a):
    W1c = (np.asarray(W1, np.float32)[None] + np.asarray(W1_a, np.float32))
    W2c = (np.asarray(W2, np.float32)[None] + np.asarray(W2_a, np.float32))
    Vc = (np.asarray(V, np.float32)[None] + np.asarray(V_a, np.float32))
    b3 = Vc[:, :, :, 0]                                  # [J, Q, O]
    slots, assign, samples, pairs = _schedule(judge_ids)
    in_maps = _pack_inputs(x, judge_ids, W1c, W2c, Vc, slots, assign, samples)
    return in_maps, (judge_ids, b3, slots, assign, samples)


# ----------------------------------------------------------------------------
# entry points
# ----------------------------------------------------------------------------

def kernel(x, judge_ids, W1, W1_a, W2, W2_a, V, V_a):
    from concourse import bass_utils
    in_maps, meta = _prepare(x, judge_ids, W1, W1_a, W2, W2_a, V, V_a)
    nc = _get_program(meta[2])
    res = bass_utils.run_bass_kernel_spmd(
        nc, in_maps, core_ids=list(range(N_CORES)), trace=False)
    return _unpack_output(res.results, meta[0], meta[1], meta[2], meta[3], meta[4])


def run_with_results(x, judge_ids, W1, W1_a, W2, W2_a, V, V_a, trace=False,
                     **kwargs):
    from concourse import bass_utils
    in_maps, meta = _prepare(x, judge_ids, W1, W1_a, W2, W2_a, V, V_a)
    nc = _get_program(meta[2])
    res = bass_utils.run_bass_kernel_spmd(
        nc, in_maps, core_ids=list(range(N_CORES)), trace=trace, **kwargs)
    out = _unpack_output(res.results, meta[0], meta[1], meta[2], meta[3], meta[4])
    return out, res
